# revision 29
# baseline (speedup 1.0000x reference)
"""Back-projection kernel for Trainium2 (8 NeuronCores) — adaptive regions.

See kernel.py docstring for the math.  This version additionally lets every
angle choose its own w-region width from {8,16,32,64,128} (wide regions for
axis-aligned angles whose u-band barely depends on w; narrow regions for
diagonal angles), minimizing streamed T+p bytes.  Bands are continuously
packed per (class, region) and may split across bins.
"""

import numpy as np
import ml_dtypes

B, NANG, L = 2, 96, 128
NA2 = NANG // 2
H = W = U = L
NCORES = 8
HPC = H // NCORES       # 16 output rows per core
BZ = B * L              # 256 (b,z) columns
PAIR = 2 * BZ           # 512 = [p_a | p_{a+48}] column block
DELTA = 1e-11
NCH = 16                # px chunks (128 px each: 16h x 8w)
CW = 8
WSOPTS = (8, 16, 32, 64, 128)
PDMA_COLS = 4608        # merge p blocks into DMAs of >= ~1.1MB

_cache = {}


def _host_maps(angles: np.ndarray):
    a = angles.astype(np.float32)
    phi = (np.float32(270.0) - a).astype(np.float32)
    th = (phi * np.float32(np.pi / 180.0)).astype(np.float32)
    c = np.cos(th).astype(np.float32)[:, None, None]
    s = np.sin(th).astype(np.float32)[:, None, None]
    cy = cx = np.float32((L - 1) / 2.0)
    hh, ww = np.meshgrid(np.arange(H, dtype=np.float32),
                         np.arange(W, dtype=np.float32), indexing="ij")
    xr = (ww - cx)[None]
    yr = (hh - cy)[None]
    sx = (c * xr + s * yr + cx).astype(np.float32)
    sy = (-s * xr + c * yr + cy).astype(np.float32)
    x0 = np.floor(sx)
    y0 = np.floor(sy)
    fx = (sx - x0).astype(np.float64)
    fy = (sy - y0).astype(np.float64)
    x0i = x0.astype(np.int64)
    y0i = y0.astype(np.int64)
    my0 = ((y0i >= 0) & (y0i < H)).astype(np.float64)
    my1 = ((y0i + 1 >= 0) & (y0i + 1 < H)).astype(np.float64)
    mx0 = ((x0i >= 0) & (x0i < W)).astype(np.float64)
    mx1 = ((x0i + 1 >= 0) & (x0i + 1 < W)).astype(np.float64)
    wyv = (1.0 - fy) * my0 + fy * my1
    W0 = wyv * (1.0 - fx) * mx0
    W1 = wyv * fx * mx1
    I0 = np.clip(x0i, 0, W - 1)
    I1 = np.clip(x0i + 1, 0, W - 1)
    return W0, W1, I0, I1


def _make_plan(angles: np.ndarray):
    W0, W1, I0, I1 = _host_maps(angles)
    norm = (W0 + W1).sum(axis=0)
    inv = (1.0 / (norm + DELTA))[None]

    T = np.zeros((NA2, U, H, W), dtype=np.float32)
    ai = np.arange(NA2)[:, None, None]
    hi = np.arange(H)[None, :, None]
    wi = np.arange(W)[None, None, :]
    sh = I0[:NA2].shape
    aib = np.broadcast_to(ai, sh)
    hib = np.broadcast_to(hi, sh)
    wib = np.broadcast_to(wi, sh)
    T[aib, I0[:NA2], hib, wib] += (W0[:NA2] * inv).astype(np.float32)
    T[aib, I1[:NA2], hib, wib] += (W1[:NA2] * inv).astype(np.float32)

    big = 999
    I0m = np.where(W0 > 0, I0, big)[:NA2]
    I1m = np.where(W1 > 0, I1, big)[:NA2]
    I0M = np.where(W0 > 0, I0, -1)[:NA2]
    I1M = np.where(W1 > 0, I1, -1)[:NA2]

    lo_ws, wd_ws = {}, {}
    for ws in WSOPTS:
        R = W // ws
        lo = np.minimum(I0m, I1m).reshape(NA2, NCORES, HPC, R, ws).min(axis=(2, 4))
        hi_ = np.maximum(I0M, I1M).reshape(NA2, NCORES, HPC, R, ws).max(axis=(2, 4))
        wd = np.where(hi_ >= 0, hi_ - np.where(lo == big, 0, lo) + 1, 0)
        lo_ws[ws] = lo                     # [a, core, R]
        wd_ws[ws] = wd.max(axis=1)         # [a, R]

    # class-subset + per-angle assignment: minimize max(DMA time, PE time).
    # bin-tail padding (ceil to 128 rows per class-region) is modeled, and a
    # local search moves angles between classes to fill bins.
    def plan_cost(subset):
        chosen_ = []
        for a in range(NA2):
            best = None
            for ws in subset:
                rows = int(wd_ws[ws][a].sum())
                cost = rows * (HPC * ws + PAIR)
                if best is None or cost < best[0]:
                    best = (cost, ws)
            chosen_.append(best[1])

        def totals(ch):
            tcols_ = pcols_ = 0
            for ws in subset:
                R = W // ws
                for r in range(R):
                    rows = sum(int(wd_ws[ws][a, r]) for a in range(NA2)
                               if ch[a] == ws)
                    nbv = -(-rows // 128)
                    tcols_ += nbv * HPC * ws
                    pcols_ += nbv * PAIR
            return tcols_, pcols_

        tc, pc = totals(chosen_)
        improved = True
        sweeps = 0
        while improved and sweeps < 4:
            improved = False
            sweeps += 1
            for a in range(NA2):
                cur = chosen_[a]
                best = (tc + pc * 1.0, cur, tc, pc)
                for ws in subset:
                    if ws == cur:
                        continue
                    chosen_[a] = ws
                    tc2, pc2 = totals(chosen_)
                    if tc2 + pc2 < best[0]:
                        best = (tc2 + pc2, ws, tc2, pc2)
                chosen_[a] = best[1]
                if best[1] != cur:
                    improved = True
                    tc, pc = best[2], best[3]
        return chosen_, tc, pc

    best_sub = None
    for subset in ((16, 32, 64), (16, 32), (16, 64), (32,), (16,),
                   (16, 32, 128)):
        chosen_, tc, pc = plan_cost(subset)
        # calibrated on traced runs: DMA path = init + transfer + slack;
        # PE path = start latency + cycles (mild ramp factor) + drain tail
        dma_ns = 2330 + (tc + pc + NCH * PAIR) * 128 * 2 / 360.0 + 1500
        pe_ns = 6500 + tc / 128 * 512 / 2.4 * 1.05 + 1200 + 1800
        score = max(dma_ns, pe_ns)
        if best_sub is None or score < best_sub[0]:
            best_sub = (score, subset, chosen_)
    _, subset, chosen = best_sub
    # ascending: narrow classes first, so chunk 0's first matmuls need only
    # the small first p blocks and the PE starts sooner
    classes = sorted(set(subset))

    # continuous packing per (class, region)
    items = {}   # (ci, r) -> list of (a, gpos, w)
    nb = {}      # (ci, r) -> bins
    for ci, ws in enumerate(classes):
        R = W // ws
        for r in range(R):
            pos = 0
            its = []
            for a in range(NA2):
                if chosen[a] != ws:
                    continue
                w_ = int(wd_ws[ws][a, r])
                if w_ <= 0:
                    continue
                its.append((a, pos, w_))
                pos += w_
            items[(ci, r)] = its
            nb[(ci, r)] = -(-pos // 128)

    # p layout in first-use order, merged into DMA pieces
    p_off = {}
    p_order = []
    pcol = 0
    for c in range(NCH):
        for ci, ws in enumerate(classes):
            r = (CW * c) // ws
            if r * ws != CW * c:
                continue            # region does not start at this chunk
            p_off[(ci, r)] = pcol
            pcol += nb[(ci, r)] * PAIR
            p_order.append((c, pcol))      # block first used by chunk c
    pcols = pcol
    # merge into DMA pieces; cut right after chunk 0's blocks so the PE can
    # start as soon as possible
    p_dmas = []
    start = 0
    for i, (c, end) in enumerate(p_order):
        nxt_c = p_order[i + 1][0] if i + 1 < len(p_order) else None
        if (end - start >= PDMA_COLS or i == len(p_order) - 1
                or (c == 0 and nxt_c is not None and nxt_c > 0)):
            p_dmas.append((start, end))
            start = end
    # p_need[c]: number of p DMA pieces chunk c requires
    p_need = []
    for c in range(NCH):
        need_col = 0
        for ci, ws in enumerate(classes):
            r = (CW * c) // ws
            need_col = max(need_col, p_off[(ci, r)] + nb[(ci, r)] * PAIR)
        n = 0
        for (s_, e_) in p_dmas:
            n += 1
            if e_ >= need_col:
                break
        p_need.append(n)

    # T layout: chunk-major [c][class][bin] blocks of 128 cols
    tbase = {}
    t_off = [0] * (NCH + 1)
    tco = 0
    for c in range(NCH):
        t_off[c] = tco
        for ci, ws in enumerate(classes):
            r = (CW * c) // ws
            for g in range(nb[(ci, r)]):
                tbase[(c, ci, g)] = tco
                tco += 128
    t_off[NCH] = tco

    # per-chunk matmul rhs column offsets (aligned with T block order), and
    # the p-DMA piece index each matmul depends on (for fine-grained waits)
    mm = []
    mm_req = []
    ends = [e for (_, e) in p_dmas]
    import bisect
    for c in range(NCH):
        lst = []
        req = []
        for ci, ws in enumerate(classes):
            r = (CW * c) // ws
            for g in range(nb[(ci, r)]):
                pc = p_off[(ci, r)] + g * PAIR
                lst.append(pc)
                req.append(bisect.bisect_right(ends, pc + PAIR - 1) + 1)
        mm.append(lst)
        mm_req.append(req)

    return {
        "T": T,
        "classes": classes,
        "chosen": chosen,
        "lo_ws": lo_ws,
        "items": items,
        "nb": nb,
        "p_off": p_off,
        "p_dmas": p_dmas,
        "p_need": p_need,
        "tbase": tbase,
        "t_off": t_off,
        "mm": mm,
        "mm_req": mm_req,
        "tcols": tco,
        "pcols": pcols,
    }


def _build_inputs(image: np.ndarray, plan):
    T = plan["T"]
    classes = plan["classes"]
    items = plan["items"]
    nbm = plan["nb"]
    p_off = plan["p_off"]
    tbase = plan["tbase"]
    p = image.transpose(2, 1, 0, 3).reshape(U, NANG, BZ)
    in_maps = []
    for core in range(NCORES):
        hs = slice(HPC * core, HPC * (core + 1))
        tpack = np.zeros((128, plan["tcols"]), dtype=ml_dtypes.bfloat16)
        ppack = np.zeros((128, plan["pcols"]), dtype=ml_dtypes.bfloat16)
        for ci, ws in enumerate(classes):
            R = W // ws
            nlc = ws // CW
            lo = plan["lo_ws"][ws]
            for r in range(R):
                c0 = (ws * r) // CW
                pc0 = p_off[(ci, r)]
                for a, gpos, w_ in items[(ci, r)]:
                    k0 = int(lo[a, core, r])
                    if k0 == 999:
                        continue
                    k0 = min(k0, 128 - w_)
                    tb = T[a, k0:k0 + w_, hs, ws * r:ws * (r + 1)]
                    blk = tb.reshape(w_, HPC, nlc, CW)
                    i = 0
                    while i < w_:
                        row = (gpos + i) % 128
                        g = (gpos + i) // 128
                        n = min(w_ - i, 128 - row)
                        for lc in range(nlc):
                            c0t = tbase[(c0 + lc, ci, g)]
                            tpack[row:row + n, c0t:c0t + 128] = \
                                blk[i:i + n, :, lc, :].reshape(n, 128)
                        pc = pc0 + g * PAIR
                        ppack[row:row + n, pc:pc + BZ] = p[k0 + i:k0 + i + n, a, :]
                        ppack[row:row + n, pc + BZ:pc + PAIR] = \
                            p[k0 + i:k0 + i + n, a + NA2, :]
                        i += n
        in_maps.append({"tmat": np.ascontiguousarray(tpack),
                        "ppack": np.ascontiguousarray(ppack)})
    return in_maps


def _build_program_raw(plan):
    import concourse.bass as bass
    import concourse.mybir as mybir

    t_off = plan["t_off"]
    mm = plan["mm"]
    p_dmas = plan["p_dmas"]
    p_need = plan["p_need"]
    maxt = max(t_off[c + 1] - t_off[c] for c in range(NCH))

    nc = bass.Bass(trn_type="TRN2")
    bf16 = mybir.dt.bfloat16
    f32 = mybir.dt.float32

    t_dram = nc.dram_tensor("tmat", [128, plan["tcols"]], bf16,
                            kind="ExternalInput")
    p_dram = nc.dram_tensor("ppack", [128, plan["pcols"]], bf16,
                            kind="ExternalInput")
    o_dram = nc.dram_tensor("out", [128, NCH * PAIR], bf16,
                            kind="ExternalOutput")

    NSLOT = 6
    NPD = len(p_dmas)
    from contextlib import ExitStack
    with ExitStack() as stack:
        ec = stack.enter_context
        # count-based waits on one semaphore are only safe when the DMAs
        # complete in issue order; hardware spreads DMAs over several rings,
        # so give every p piece its own semaphore and every T slot its own
        # (slot reuse is causally ordered through s_mm, so per-slot counting
        # is safe).
        s_pps = [ec(nc.semaphore(f"s_pp{i}")) for i in range(NPD)]
        s_ts = [ec(nc.semaphore(f"s_t{i}")) for i in range(NSLOT)]
        s_mm = ec(nc.semaphore("s_mm"))
        s_cp = ec(nc.semaphore("s_cp"))
        s_out = ec(nc.semaphore("s_out"))
        pp_sb = ec(nc.sbuf_tensor("pp_sb", [128, plan["pcols"]], bf16))
        t_slots = [ec(nc.sbuf_tensor(f"t_sb{i}", [128, maxt], bf16))
                   for i in range(NSLOT)]
        o_all = ec(nc.sbuf_tensor("o_all", [128, NCH * PAIR], bf16))
        psums = [ec(nc.psum_tensor(f"ps{i}", [128, 512], f32))
                 for i in range(4)]
        ps_dummy = ec(nc.psum_tensor("ps_dummy", [128, 512], f32))
        banks = [ps_[:, :512] for ps_ in psums]

        with nc.Block() as block:

            @block.sync
            def _(sync):
                # SP: p pieces (no waits, so they always stream ahead),
                # then progressively finer out stores
                for pi in range(NPD):
                    lo_, hi_ = p_dmas[pi]
                    sync.dma_start(
                        pp_sb[:, lo_:hi_], p_dram[:, lo_:hi_]
                    ).then_inc(s_pps[pi], 16)
                store_at = [0, 6, 12, 15, 16]
                for k in range(4):
                    sync.wait_ge(s_cp, store_at[k + 1])
                    sync.dma_start(
                        o_dram[:, store_at[k] * PAIR:store_at[k + 1] * PAIR],
                        o_all[:, store_at[k] * PAIR:store_at[k + 1] * PAIR],
                    ).then_inc(s_out, 16)
                sync.wait_ge(s_out, 64)

            @block.scalar
            def _(scalar):
                # Activation engine: T loads with slot recycling; its waits
                # must not stall the SP p stream
                for c in range(NCH):
                    if c >= NSLOT:
                        scalar.wait_ge(s_mm, c - NSLOT + 1)
                    nt = t_off[c + 1] - t_off[c]
                    scalar.dma_start(
                        t_slots[c % NSLOT][:, :nt],
                        t_dram[:, t_off[c]:t_off[c + 1]],
                    ).then_inc(s_ts[c % NSLOT], 16)

            @block.tensor
            def _(tensor):
                mm_req = plan["mm_req"]
                seen_p = 0
                for c in range(NCH):
                    nbk = len(mm[c])
                    tensor.wait_ge(s_ts[c % NSLOT], 16 * (c // NSLOT + 1))
                    if c >= 4:
                        tensor.wait_ge(s_cp, c - 3)
                    ps = banks[c % 4]
                    t_sb = t_slots[c % NSLOT]
                    for i, pc in enumerate(mm[c]):
                        while seen_p < mm_req[c][i]:
                            tensor.wait_ge(s_pps[seen_p], 16)
                            seen_p += 1
                        mmi = tensor.matmul(
                            ps,
                            t_sb[:, i * 128:(i + 1) * 128],
                            pp_sb[:, pc:pc + PAIR],
                            start=(i == 0),
                            stop=(i == nbk - 1),
                        )
                        if i == nbk - 1:
                            mmi.then_inc(s_mm, 1)
                tensor.matmul(
                    ps_dummy[:, :BZ],
                    pp_sb[:, :128],
                    pp_sb[:, :BZ],
                    start=True,
                    stop=True,
                ).then_inc(s_mm, 1)

            @block.vector
            def _(vector):
                for c in range(NCH):
                    vector.wait_ge(s_mm, c + 2)
                    vector.tensor_copy(
                        o_all[:, c * PAIR:(c + 1) * PAIR], banks[c % 4]
                    ).then_inc(s_cp, 1)

    nc.finalize()
    return nc


def kernel(image: np.ndarray, angles: np.ndarray) -> np.ndarray:
    from concourse.bass_utils import run_bass_kernel_spmd

    image = np.ascontiguousarray(image, dtype=np.float32)
    angles = np.ascontiguousarray(angles, dtype=np.float32)

    key = angles.tobytes()
    if key not in _cache:
        plan = _make_plan(angles)
        nc = _build_program_raw(plan)
        _cache[key] = (nc, plan)
    nc, plan = _cache[key]

    in_maps = _build_inputs(image, plan)
    res = run_bass_kernel_spmd(nc, in_maps, core_ids=list(range(NCORES)))

    acc = np.empty((2, B, H, W, L), dtype=np.float32)
    for core in range(NCORES):
        o = np.asarray(res.results[core]["out"]).astype(np.float32)
        # cols = chunk(16) * [acc1 256 | acc2 256] ; px = hl*8+wl ; w = c*8+wl
        o = o.reshape(HPC, CW, NCH, 2, B, L)       # [hl, wl, c, acc, b, z]
        o = o.transpose(3, 4, 0, 2, 1, 5)          # [acc, b, hl, c, wl, z]
        acc[:, :, HPC * core:HPC * (core + 1)] = o.reshape(2, B, HPC, W, L)
    out = acc[0] + acc[1][:, ::-1, ::-1, :]
    return np.ascontiguousarray(out, dtype=np.float32)
